# revision 14
# baseline (speedup 1.0000x reference)
"""Trainium2 Bass kernel for the ANEAttention problem (GQA attention block).

Reference computation (per batch b):
    q = Wq @ Xq[b]          -> [H*D, S], RoPE applied per head
    k = Wk @ Xkv[b]         -> [D, S],   RoPE applied (single KV head)
    v = Wv @ Xkv[b]         -> [D, S]
    scores = (q_h . k) / sqrt(D)   (attn_mask is all zeros per the spec)
    probs  = softmax over k
    out    = Wo @ concat_h(probs @ v^T)

Sharding: B=2 batches x 4 query-sequence blocks = 8 cores.  Each core
computes all heads for its 512 query positions, so the output projection
contracts over all heads locally and each core emits a disjoint
[2048, 512] slice of the final output.  K/V projections are sharded the
same way (each core projects its own 512 k-positions) and AllGathered
(one fused collective) across the 4 cores of the batch group, overlapped
with the Q projection.

All matmuls run in bf16 (f32 PSUM accumulate); softmax runs in f32 via
ScalarE exp.  Weights are pre-transposed on the host so every matmul
operand is a natural [contraction-on-partition] SBUF tile.  Softmax skips
the max-subtraction: scores are bounded (|s| < ~8) by construction, so
exp cannot overflow f32.

The scores scale 1/sqrt(D) is folded into sin_q/cos_q on the host.

Scheduling notes (hard-won):
  - Two HWDGE rings (sync + scalar) stream weights/activations as 2-D
    chunk DMAs; DMAs that wait on the collective sit at the ring tails.
  - The Wq streaming pool is allocated BEFORE the K/V input pool so its
    tiles do not reuse that pool's SBUF (a reuse would make the Wq DMAs
    wait for the last K/V matmul).
  - Attention is software-pipelined by one head; the attnT->attn
    transposes ride between head matmuls so their LDWEIGHTS stay hidden.
"""

from contextlib import ExitStack

import numpy as np
import ml_dtypes

P = 128
B = 2
HID = 2048
S = 2048
H = 8
D = 256
SB = 512               # per-core query/key block length (S / 4)
NCT = HID // P         # 16 contraction tiles over hidden
VTW = D + 1            # V^T tile width: 256 cols of V^T plus a ones column
GROUPS = [[0, 1, 2, 3], [4, 5, 6, 7]]   # batch groups (core = b*4 + j)
KP = P * SB            # elems per [128, 512] block in the flat AG bounce
VP = P * D
SHARD = 2 * KP + 4 * VP  # per-rank AG shard: K part then V^T part

BF16 = ml_dtypes.bfloat16

_CACHE = {}


def _rope(nc, pool, f32, p1, p2, sin, cos, out1, out2, w, uid):
    """out1 = p1*cos - p2*sin ; out2 = p2*cos + p1*sin (DVE, f32 -> bf16)."""
    t1 = pool.tile([P, w], f32, tag="t1", name=f"t1_{uid}")
    t2 = pool.tile([P, w], f32, tag="t2", name=f"t2_{uid}")
    t3 = pool.tile([P, w], f32, tag="t3", name=f"t3_{uid}")
    t4 = pool.tile([P, w], f32, tag="t4", name=f"t4_{uid}")
    nc.vector.tensor_mul(t1[:], p1[:], cos)
    nc.vector.tensor_mul(t2[:], p2[:], sin)
    nc.vector.tensor_sub(out1, t1[:], t2[:])
    nc.vector.tensor_mul(t3[:], p2[:], cos)
    nc.vector.tensor_mul(t4[:], p1[:], sin)
    nc.vector.tensor_add(out2, t3[:], t4[:])


def _build():
    import concourse.mybir as mybir
    import concourse.tile as tile
    from concourse import bacc
    from concourse.masks import make_identity

    bf = mybir.dt.bfloat16
    f32 = mybir.dt.float32
    Exp = mybir.ActivationFunctionType.Exp

    nc = bacc.Bacc("TRN2", target_bir_lowering=False, debug=False, num_devices=8)

    xq_d = nc.declare_dram_parameter("xq", [HID, SB], bf, isOutput=False)
    xkv_d = nc.declare_dram_parameter("xkv", [HID, SB], bf, isOutput=False)
    wq_d = nc.declare_dram_parameter("wqT", [HID, H * D], bf, isOutput=False)
    wk_d = nc.declare_dram_parameter("wkT", [HID, D], bf, isOutput=False)
    wv_d = nc.declare_dram_parameter("wvT", [HID, D], bf, isOutput=False)
    wo_d = nc.declare_dram_parameter("woT", [H * D, HID], bf, isOutput=False)
    sinq_d = nc.declare_dram_parameter("sinq", [D // 2, SB], f32, isOutput=False)
    cosq_d = nc.declare_dram_parameter("cosq", [D // 2, SB], f32, isOutput=False)
    sink_d = nc.declare_dram_parameter("sink", [D // 2, SB], f32, isOutput=False)
    cosk_d = nc.declare_dram_parameter("cosk", [D // 2, SB], f32, isOutput=False)
    out_d = nc.declare_dram_parameter("out", [HID, SB], f32, isOutput=True)

    with tile.TileContext(nc) as tc, ExitStack() as es:
        constp = es.enter_context(tc.tile_pool(name="const", bufs=1))
        persist = es.enter_context(tc.tile_pool(name="persist", bufs=1))
        dram = es.enter_context(tc.tile_pool(name="dram", bufs=1, space="DRAM"))
        # Wq streaming pool first, so its slots never alias the K/V pool.
        # Released manually right after the Q projection.
        wqp = tc.alloc_tile_pool(name="wqp", bufs=4)

        ident = constp.tile([P, P], bf, name="ident")
        make_identity(nc, ident[:])
        sinq = constp.tile([P, SB], f32, name="sinq")
        cosq = constp.tile([P, SB], f32, name="cosq")
        sink = constp.tile([P, SB], f32, name="sink")
        cosk = constp.tile([P, SB], f32, name="cosk")

        # Persistent per-core intermediates (bf16, [part, free]):
        q_sb = persist.tile([P, 16 * SB], bf, name="q_sb")      # Q rows (h,d)
        k_sb = persist.tile([P, 2 * S], bf, name="k_sb")        # K, 2 d-half tiles
        vt_sb = persist.tile([P, 16 * VTW], bf, name="vt_sb")   # V^T k-tiles + ones

        kin_b = dram.tile([2 * KP], bf, name="kin_b")
        kout_b = dram.tile([8 * KP], bf, name="kout_b")
        vin_b = dram.tile([4 * VP], bf, name="vin_b")
        vout_b = dram.tile([16 * VP], bf, name="vout_b")

        # ---- Phase 1: local K and V^T projections (this core's 512
        # k-positions), then one fused AllGather per batch group ----
        with tc.tile_pool(name="kvin", bufs=1) as kvin, \
             tc.tile_pool(name="kvloc", bufs=1) as kvloc, \
             tc.tile_pool(name="psk", bufs=2, space="PSUM") as psk, \
             tc.tile_pool(name="psv", bufs=2, space="PSUM") as psv, \
             tc.tile_pool(name="ropek", bufs=1) as ropek:
            wk_sb = kvin.tile([P, NCT * D], bf, name="wk_sb")
            xkv_sb = kvin.tile([P, NCT * SB], bf, name="xkv_sb")
            wv_sb = kvin.tile([P, NCT * D], bf, name="wv_sb")
            for ct in range(NCT):
                e1, e2 = (nc.sync, nc.scalar) if ct % 2 == 0 else (nc.scalar, nc.sync)
                e1.dma_start(out=wk_sb[:, ct * D:(ct + 1) * D],
                             in_=wk_d[ct * P:(ct + 1) * P, :])
                e2.dma_start(out=xkv_sb[:, ct * SB:(ct + 1) * SB],
                             in_=xkv_d[ct * P:(ct + 1) * P, :])
            nc.sync.dma_start(out=sink[:], in_=sink_d[:, :])
            nc.scalar.dma_start(out=cosk[:], in_=cosk_d[:, :])
            for ct in range(NCT):
                eng = nc.sync if ct % 2 == 0 else nc.scalar
                eng.dma_start(out=wv_sb[:, ct * D:(ct + 1) * D],
                              in_=wv_d[ct * P:(ct + 1) * P, :])
            nc.gpsimd.memset(vt_sb[:], 1.0)  # ones column survives the V copies

            # local K proj + RoPE
            k_loc = kvloc.tile([P, 2 * SB], bf, name="k_loc")
            pk1 = psk.tile([P, SB], f32, tag="pk", name="pk1")
            pk2 = psk.tile([P, SB], f32, tag="pk", name="pk2")
            for ct in range(NCT):
                nc.tensor.matmul(pk1[:], wk_sb[:, ct * D:ct * D + P],
                                 xkv_sb[:, ct * SB:(ct + 1) * SB],
                                 start=(ct == 0), stop=(ct == NCT - 1))
            for ct in range(NCT):
                nc.tensor.matmul(pk2[:], wk_sb[:, ct * D + P:ct * D + 2 * P],
                                 xkv_sb[:, ct * SB:(ct + 1) * SB],
                                 start=(ct == 0), stop=(ct == NCT - 1))
            _rope(nc, ropek, f32, pk1, pk2, sink[:], cosk[:],
                  k_loc[:, 0:SB], k_loc[:, SB:2 * SB], SB, "k")
            nc.gpsimd.dma_start(out=kin_b[0:KP].rearrange("(p c) -> p c", c=SB),
                                in_=k_loc[:, 0:SB])
            nc.gpsimd.dma_start(out=kin_b[KP:2 * KP].rearrange("(p c) -> p c", c=SB),
                                in_=k_loc[:, SB:2 * SB])
            nc.gpsimd.collective_compute(
                "AllGather", mybir.AluOpType.bypass,
                ins=[kin_b[:].opt()], outs=[kout_b[:].opt()],
                replica_groups=GROUPS)

            # local V^T proj
            vt_loc = kvloc.tile([P, 4 * D], bf, name="vt_loc")
            for st in range(4):
                pv = psv.tile([P, D], f32, tag="pv", name=f"pv_{st}")
                for ct in range(NCT):
                    nc.tensor.matmul(pv[:],
                                     xkv_sb[:, ct * SB + st * P:ct * SB + (st + 1) * P],
                                     wv_sb[:, ct * D:(ct + 1) * D],
                                     start=(ct == 0), stop=(ct == NCT - 1))
                nc.vector.tensor_copy(vt_loc[:, st * D:(st + 1) * D], pv[:])
            for st in range(4):
                nc.gpsimd.dma_start(
                    out=vin_b[st * VP:(st + 1) * VP].rearrange("(p c) -> p c", c=D),
                    in_=vt_loc[:, st * D:(st + 1) * D])
            nc.gpsimd.collective_compute(
                "AllGather", mybir.AluOpType.bypass,
                ins=[vin_b[:].opt()], outs=[vout_b[:].opt()],
                replica_groups=GROUPS)

        # ---- Phase 2: Q projection + RoPE (Wq streamed in 4 quarters) ----
        with tc.tile_pool(name="qin", bufs=1) as qin, \
             tc.tile_pool(name="psq", bufs=4, space="PSUM") as psq, \
             tc.tile_pool(name="ropeq", bufs=2) as ropeq:
            xq_sb = qin.tile([P, NCT * SB], bf, name="xq_sb")
            nc.sync.dma_start(out=sinq[:], in_=sinq_d[:, :])
            nc.scalar.dma_start(out=cosq[:], in_=cosq_d[:, :])
            wq_quarters = []
            for quarter in range(4):
                wqq = wqp.tile([P, NCT * SB], bf, tag="wqq", name=f"wqq_{quarter}")
                wq_quarters.append(wqq)
                for ct in range(NCT):
                    eng = nc.sync if ct % 2 == 0 else nc.scalar
                    eng.dma_start(
                        out=wqq[:, ct * SB:(ct + 1) * SB],
                        in_=wq_d[ct * P:(ct + 1) * P,
                                 quarter * SB:(quarter + 1) * SB])
                if quarter == 0:
                    # xq streams after the first weight quarter: both are
                    # needed at the same moment, but the weight gates the
                    # whole quarter while xq chunks gate one matmul each.
                    for ct in range(NCT):
                        eng = nc.sync if ct % 2 == 0 else nc.scalar
                        eng.dma_start(out=xq_sb[:, ct * SB:(ct + 1) * SB],
                                      in_=xq_d[ct * P:(ct + 1) * P, :])
                for hh in range(2):
                    h = quarter * 2 + hh
                    pq1 = psq.tile([P, SB], f32, tag="pq", name=f"pq1_{h}")
                    pq2 = psq.tile([P, SB], f32, tag="pq", name=f"pq2_{h}")
                    for ct in range(NCT):
                        nc.tensor.matmul(pq1[:],
                                         wqq[:, ct * SB + 2 * hh * P:ct * SB + (2 * hh + 1) * P],
                                         xq_sb[:, ct * SB:(ct + 1) * SB],
                                         start=(ct == 0), stop=(ct == NCT - 1))
                    for ct in range(NCT):
                        nc.tensor.matmul(pq2[:],
                                         wqq[:, ct * SB + (2 * hh + 1) * P:ct * SB + (2 * hh + 2) * P],
                                         xq_sb[:, ct * SB:(ct + 1) * SB],
                                         start=(ct == 0), stop=(ct == NCT - 1))
                    _rope(nc, ropeq, f32, pq1, pq2, sinq[:], cosq[:],
                          q_sb[:, 2 * h * SB:(2 * h + 1) * SB],
                          q_sb[:, (2 * h + 1) * SB:(2 * h + 2) * SB], SB, f"q{h}")

            # ring-tail unpacks of the gathered K/V shards (each waits on
            # the collective; nothing streams behind them except Wo)
            for j in range(4):
                for dt in range(2):
                    eng = nc.sync if dt == 0 else nc.scalar
                    eng.dma_start(
                        out=k_sb[:, dt * S + j * SB:dt * S + (j + 1) * SB],
                        in_=kout_b[(2 * j + dt) * KP:(2 * j + dt + 1) * KP]
                            .rearrange("(p c) -> p c", c=SB))
            for gst in range(16):
                eng = nc.sync if gst % 2 == 0 else nc.scalar
                eng.dma_start(
                    out=vt_sb[:, gst * VTW:gst * VTW + D],
                    in_=vout_b[gst * VP:(gst + 1) * VP].rearrange("(p c) -> p c", c=D))
        wqp.release()

        # ---- Phase 3+4: attention, software-pipelined by one head ----
        with tc.tile_pool(name="wop", bufs=2) as wop, \
             tc.tile_pool(name="attnp", bufs=1) as attnp:
            attnT = attnp.tile([P, 4 * H * D], bf, name="attnT")
            attn = attnp.tile([P, 16 * SB], bf, name="attn")
            wo_tiles = []
            for half in range(2):
                woh = wop.tile([P, NCT * 1024], bf, tag="woh", name=f"woh_{half}")
                wo_tiles.append(woh)
                for c2 in range(NCT):
                    eng = nc.sync if c2 % 2 == 0 else nc.scalar
                    eng.dma_start(
                        out=woh[:, c2 * 1024:(c2 + 1) * 1024],
                        in_=wo_d[c2 * P:(c2 + 1) * P, half * 1024:(half + 1) * 1024])

            attention_pools = (
                tc.tile_pool(name="expp", bufs=2),
                tc.tile_pool(name="pss", bufs=2, space="PSUM"),
                tc.tile_pool(name="psa", bufs=2, space="PSUM"),
                tc.tile_pool(name="pst", bufs=2, space="PSUM"),
                tc.tile_pool(name="smallp", bufs=4),
            )
            attn_es = ExitStack()
            expp, pss, psa, pst, smallp = (attn_es.enter_context(p)
                                           for p in attention_pools)

            exp_tiles = {}

            def scores_head(h):
                q0 = q_sb[:, 2 * h * SB:(2 * h + 1) * SB]
                q1 = q_sb[:, (2 * h + 1) * SB:(2 * h + 2) * SB]
                expT = expp.tile([P, 16 * SB], bf, tag="expT", name=f"expT_{h}")
                exp_tiles[h] = expT
                for kp in range(8):  # pairs of k-tiles -> one [128,1024] exp
                    ps = pss.tile([P, 2 * SB], f32, tag="ps", name=f"ps_{h}_{kp}")
                    for i in range(2):
                        kt = 2 * kp + i
                        nc.tensor.matmul(ps[:, i * SB:(i + 1) * SB],
                                         k_sb[:, kt * P:(kt + 1) * P], q0,
                                         start=True, stop=False)
                        nc.tensor.matmul(ps[:, i * SB:(i + 1) * SB],
                                         k_sb[:, S + kt * P:S + (kt + 1) * P], q1,
                                         start=False, stop=True)
                    nc.scalar.activation(expT[:, 2 * kp * SB:2 * (kp + 1) * SB],
                                         ps[:], Exp)

            def attnout_head(h):
                expT = exp_tiles.pop(h)
                for qt in range(4):
                    pa = psa.tile([P, VTW], f32, tag="pa", name=f"pa_{h}_{qt}")
                    for kt in range(16):
                        nc.tensor.matmul(pa[:],
                                         expT[:, kt * SB + qt * P:kt * SB + (qt + 1) * P],
                                         vt_sb[:, kt * VTW:(kt + 1) * VTW],
                                         start=(kt == 0), stop=(kt == 15))
                    rcp = smallp.tile([P, 1], f32, tag="rcp", name=f"rcp_{h}_{qt}")
                    nc.vector.reciprocal(rcp[:], pa[:, D:D + 1])
                    nc.vector.tensor_scalar_mul(
                        attnT[:, qt * H * D + h * D:qt * H * D + (h + 1) * D],
                        pa[:, 0:D], rcp[:])
                # transpose this head's attnT tiles into attn ([c', q])
                for qt in range(4):
                    for c2 in (2 * h, 2 * h + 1):
                        ptr = pst.tile([P, P], bf, tag="ptr", name=f"ptr_{h}_{qt}_{c2}")
                        nc.tensor.transpose(
                            ptr[:],
                            attnT[:, qt * H * D + c2 * P:qt * H * D + (c2 + 1) * P],
                            ident[:])
                        nc.vector.tensor_copy(
                            attn[:, c2 * SB + qt * P:c2 * SB + (qt + 1) * P], ptr[:])

            scores_head(0)
            for h in range(1, H):
                scores_head(h)
                attnout_head(h - 1)
            attnout_head(H - 1)
            attn_es.close()  # free attention PSUM banks before phase 5

            # ---- Phase 5: output projection ----
            with tc.tile_pool(name="pso", bufs=2, space="PSUM") as pso, \
                 tc.tile_pool(name="outp", bufs=3) as outp:
                for half in range(2):
                    woh = wo_tiles[half]
                    for oi in range(8):
                        ot = half * 8 + oi
                        po = pso.tile([P, SB], f32, tag="po", name=f"po_{ot}")
                        for c2 in range(NCT):
                            nc.tensor.matmul(
                                po[:],
                                woh[:, c2 * 1024 + oi * P:c2 * 1024 + (oi + 1) * P],
                                attn[:, c2 * SB:(c2 + 1) * SB],
                                start=(c2 == 0), stop=(c2 == 15))
                        osb = outp.tile([P, SB], f32, tag="osb", name=f"osb_{ot}")
                        nc.scalar.copy(osb[:], po[:])
                        nc.sync.dma_start(out=out_d[ot * P:(ot + 1) * P, :],
                                          in_=osb[:])

    nc.compile()
    return nc


def _get_nc():
    if "nc" not in _CACHE:
        _CACHE["nc"] = _build()
    return _CACHE["nc"]


def make_in_maps(inputs):
    Xq = np.asarray(inputs["Xq"], np.float32)
    Xkv = np.asarray(inputs["Xkv"], np.float32)
    sin_q = np.asarray(inputs["sin_q"], np.float32)
    cos_q = np.asarray(inputs["cos_q"], np.float32)
    sin_k = np.asarray(inputs["sin_k"], np.float32)
    cos_k = np.asarray(inputs["cos_k"], np.float32)
    Wq = np.asarray(inputs["Wq"], np.float32)
    Wk = np.asarray(inputs["Wk"], np.float32)
    Wv = np.asarray(inputs["Wv"], np.float32)
    Wo = np.asarray(inputs["Wo"], np.float32)
    # attn_mask is all zeros by construction (spec fill=zeros) -> no-op.

    scale = np.float32(1.0) / np.sqrt(np.float32(D))
    wqT = np.ascontiguousarray(Wq.T).astype(BF16)
    wkT = np.ascontiguousarray(Wk.T).astype(BF16)
    wvT = np.ascontiguousarray(Wv.T).astype(BF16)
    woT = np.ascontiguousarray(Wo.T).astype(BF16)
    xq_bf = Xq.astype(BF16)
    xkv_bf = Xkv.astype(BF16)
    sinq_s = sin_q * scale
    cosq_s = cos_q * scale

    in_maps = []
    for core in range(8):
        b, j = divmod(core, 4)
        sl = slice(j * SB, (j + 1) * SB)
        in_maps.append({
            "xq": np.ascontiguousarray(xq_bf[b][:, sl]),
            "xkv": np.ascontiguousarray(xkv_bf[b][:, sl]),
            "wqT": wqT, "wkT": wkT, "wvT": wvT, "woT": woT,
            "sinq": np.ascontiguousarray(sinq_s[b, 0][:, sl]),
            "cosq": np.ascontiguousarray(cosq_s[b, 0][:, sl]),
            "sink": np.ascontiguousarray(sin_k[b, 0][:, sl]),
            "cosk": np.ascontiguousarray(cos_k[b, 0][:, sl]),
        })
    return in_maps


def kernel(**inputs):
    from concourse.bass_utils import run_bass_kernel_spmd

    nc = _get_nc()
    in_maps = make_in_maps(inputs)
    res = run_bass_kernel_spmd(nc, in_maps, core_ids=list(range(8)))
    out = np.empty((B, HID, S), np.float32)
    for core in range(8):
        b, j = divmod(core, 4)
        out[b][:, j * SB:(j + 1) * SB] = res.results[core]["out"]
    return out


# revision 16
# speedup vs baseline: 1.0688x; 1.0688x over previous
"""Trainium2 Bass kernel for the ANEAttention problem (GQA attention block).

Reference computation (per batch b):
    q = Wq @ Xq[b]          -> [H*D, S], RoPE applied per head
    k = Wk @ Xkv[b]         -> [D, S],   RoPE applied (single KV head)
    v = Wv @ Xkv[b]         -> [D, S]
    scores = (q_h . k) / sqrt(D)   (attn_mask is all zeros per the spec)
    probs  = softmax over k
    out    = Wo @ concat_h(probs @ v^T)

Sharding: B=2 batches x 4 query-sequence blocks = 8 cores.  Each core
computes all heads for its 512 query positions, so the output projection
contracts over all heads locally and each core emits a disjoint
[2048, 512] slice of the final output.  K/V projections are sharded the
same way (each core projects its own 512 k-positions) and AllGathered
(one fused collective) across the 4 cores of the batch group, overlapped
with the Q projection.

All matmuls run in bf16 (f32 PSUM accumulate); softmax runs in f32 via
ScalarE exp.  Weights are pre-transposed on the host so every matmul
operand is a natural [contraction-on-partition] SBUF tile.  Softmax skips
the max-subtraction: scores are bounded (|s| < ~8) by construction, so
exp cannot overflow f32.

The scores scale 1/sqrt(D) is folded into sin_q/cos_q on the host.

Scheduling notes (hard-won):
  - Two HWDGE rings (sync + scalar) stream weights/activations as 2-D
    chunk DMAs; DMAs that wait on the collective sit at the ring tails.
  - The Wq streaming pool is allocated BEFORE the K/V input pool so its
    tiles do not reuse that pool's SBUF (a reuse would make the Wq DMAs
    wait for the last K/V matmul).
  - Attention is software-pipelined by one head; the attnT->attn
    transposes ride between head matmuls so their LDWEIGHTS stay hidden.
"""

from contextlib import ExitStack

import numpy as np
import ml_dtypes

P = 128
B = 2
HID = 2048
S = 2048
H = 8
D = 256
SB = 512               # per-core query/key block length (S / 4)
NCT = HID // P         # 16 contraction tiles over hidden
VTW = D + 1            # V^T tile width: 256 cols of V^T plus a ones column
GROUPS = [[0, 1, 2, 3], [4, 5, 6, 7]]   # batch groups (core = b*4 + j)
KP = P * SB            # elems per [128, 512] block in the flat AG bounce
VP = P * D
SHARD = 2 * KP + 4 * VP  # per-rank AG shard: K part then V^T part

BF16 = ml_dtypes.bfloat16

_CACHE = {}


def _rope(nc, pool, f32, p1, p2, sin, cos, out1, out2, w, uid):
    """out1 = p1*cos - p2*sin ; out2 = p2*cos + p1*sin (DVE, f32 -> bf16)."""
    t1 = pool.tile([P, w], f32, tag="t1", name=f"t1_{uid}")
    t2 = pool.tile([P, w], f32, tag="t2", name=f"t2_{uid}")
    t3 = pool.tile([P, w], f32, tag="t3", name=f"t3_{uid}")
    t4 = pool.tile([P, w], f32, tag="t4", name=f"t4_{uid}")
    nc.vector.tensor_mul(t1[:], p1[:], cos)
    nc.vector.tensor_mul(t2[:], p2[:], sin)
    nc.vector.tensor_sub(out1, t1[:], t2[:])
    nc.vector.tensor_mul(t3[:], p2[:], cos)
    nc.vector.tensor_mul(t4[:], p1[:], sin)
    nc.vector.tensor_add(out2, t3[:], t4[:])


def _build():
    import concourse.mybir as mybir
    import concourse.tile as tile
    from concourse import bacc
    from concourse.masks import make_identity

    bf = mybir.dt.bfloat16
    f32 = mybir.dt.float32
    Exp = mybir.ActivationFunctionType.Exp

    nc = bacc.Bacc("TRN2", target_bir_lowering=False, debug=False, num_devices=8)

    # All inputs arrive pre-tiled as SBUF images ([P, free] with the exact
    # on-chip free layout, grouped on axis 0 for arrival granularity) so
    # every DMA row is a >=8KB contiguous descriptor (full DMA rate).
    xq_d = nc.declare_dram_parameter("xq", [4, P, 4 * SB], bf, isOutput=False)
    xkv_d = nc.declare_dram_parameter("xkv", [4, P, 4 * SB], bf, isOutput=False)
    wq_d = nc.declare_dram_parameter("wqT", [4, P, NCT * SB], bf, isOutput=False)
    wk_d = nc.declare_dram_parameter("wkT", [P, NCT * D], bf, isOutput=False)
    wv_d = nc.declare_dram_parameter("wvT", [P, NCT * D], bf, isOutput=False)
    wo_d = nc.declare_dram_parameter("woT", [2, P, NCT * 1024], bf, isOutput=False)
    sinq_d = nc.declare_dram_parameter("sinq", [D // 2, SB], f32, isOutput=False)
    cosq_d = nc.declare_dram_parameter("cosq", [D // 2, SB], f32, isOutput=False)
    sink_d = nc.declare_dram_parameter("sink", [D // 2, SB], f32, isOutput=False)
    cosk_d = nc.declare_dram_parameter("cosk", [D // 2, SB], f32, isOutput=False)
    out_d = nc.declare_dram_parameter("out", [HID, SB], f32, isOutput=True)

    with tile.TileContext(nc) as tc, ExitStack() as es:
        constp = es.enter_context(tc.tile_pool(name="const", bufs=1))
        persist = es.enter_context(tc.tile_pool(name="persist", bufs=1))
        dram = es.enter_context(tc.tile_pool(name="dram", bufs=1, space="DRAM"))
        # Wq streaming pool first, so its slots never alias the K/V pool.
        # Released manually right after the Q projection.
        wqp = tc.alloc_tile_pool(name="wqp", bufs=4)

        ident = constp.tile([P, P], bf, name="ident")
        make_identity(nc, ident[:])
        sinq = constp.tile([P, SB], f32, name="sinq")
        cosq = constp.tile([P, SB], f32, name="cosq")
        sink = constp.tile([P, SB], f32, name="sink")
        cosk = constp.tile([P, SB], f32, name="cosk")

        # Persistent per-core intermediates (bf16, [part, free]):
        q_sb = persist.tile([P, 16 * SB], bf, name="q_sb")      # Q rows (h,d)
        k_sb = persist.tile([P, 2 * S], bf, name="k_sb")        # K, 2 d-half tiles
        vt_sb = persist.tile([P, 16 * VTW], bf, name="vt_sb")   # V^T k-tiles + ones

        kin_b = dram.tile([2 * KP], bf, name="kin_b")
        kout_b = dram.tile([8 * KP], bf, name="kout_b")
        vin_b = dram.tile([4 * VP], bf, name="vin_b")
        vout_b = dram.tile([16 * VP], bf, name="vout_b")

        # ---- Phase 1: local K and V^T projections (this core's 512
        # k-positions), then one fused AllGather per batch group ----
        with tc.tile_pool(name="kvin", bufs=1) as kvin, \
             tc.tile_pool(name="kvloc", bufs=1) as kvloc, \
             tc.tile_pool(name="psk", bufs=2, space="PSUM") as psk, \
             tc.tile_pool(name="psv", bufs=2, space="PSUM") as psv, \
             tc.tile_pool(name="ropek", bufs=1) as ropek:
            wk_sb = kvin.tile([P, NCT * D], bf, name="wk_sb")
            xkv_sb = kvin.tile([P, NCT * SB], bf, name="xkv_sb")
            wv_sb = kvin.tile([P, NCT * D], bf, name="wv_sb")
            nc.sync.dma_start(out=wk_sb[:, :], in_=wk_d[:, :])
            for g in range(4):
                eng = nc.scalar if g % 2 == 0 else nc.sync
                eng.dma_start(out=xkv_sb[:, g * 4 * SB:(g + 1) * 4 * SB],
                              in_=xkv_d[g])
            nc.sync.dma_start(out=sink[:], in_=sink_d[:, :])
            nc.scalar.dma_start(out=cosk[:], in_=cosk_d[:, :])
            nc.scalar.dma_start(out=wv_sb[:, :], in_=wv_d[:, :])
            nc.gpsimd.memset(vt_sb[:], 1.0)  # ones column survives the V copies

            # local K proj + RoPE
            k_loc = kvloc.tile([P, 2 * SB], bf, name="k_loc")
            pk1 = psk.tile([P, SB], f32, tag="pk", name="pk1")
            pk2 = psk.tile([P, SB], f32, tag="pk", name="pk2")
            for ct in range(NCT):
                nc.tensor.matmul(pk1[:], wk_sb[:, ct * D:ct * D + P],
                                 xkv_sb[:, ct * SB:(ct + 1) * SB],
                                 start=(ct == 0), stop=(ct == NCT - 1))
            for ct in range(NCT):
                nc.tensor.matmul(pk2[:], wk_sb[:, ct * D + P:ct * D + 2 * P],
                                 xkv_sb[:, ct * SB:(ct + 1) * SB],
                                 start=(ct == 0), stop=(ct == NCT - 1))
            _rope(nc, ropek, f32, pk1, pk2, sink[:], cosk[:],
                  k_loc[:, 0:SB], k_loc[:, SB:2 * SB], SB, "k")
            nc.gpsimd.dma_start(out=kin_b[0:KP].rearrange("(p c) -> p c", c=SB),
                                in_=k_loc[:, 0:SB])
            nc.gpsimd.dma_start(out=kin_b[KP:2 * KP].rearrange("(p c) -> p c", c=SB),
                                in_=k_loc[:, SB:2 * SB])
            nc.gpsimd.collective_compute(
                "AllGather", mybir.AluOpType.bypass,
                ins=[kin_b[:].opt()], outs=[kout_b[:].opt()],
                replica_groups=GROUPS)

            # local V^T proj
            vt_loc = kvloc.tile([P, 4 * D], bf, name="vt_loc")
            for st in range(4):
                pv = psv.tile([P, D], f32, tag="pv", name=f"pv_{st}")
                for ct in range(NCT):
                    nc.tensor.matmul(pv[:],
                                     xkv_sb[:, ct * SB + st * P:ct * SB + (st + 1) * P],
                                     wv_sb[:, ct * D:(ct + 1) * D],
                                     start=(ct == 0), stop=(ct == NCT - 1))
                nc.vector.tensor_copy(vt_loc[:, st * D:(st + 1) * D], pv[:])
            for st in range(4):
                nc.gpsimd.dma_start(
                    out=vin_b[st * VP:(st + 1) * VP].rearrange("(p c) -> p c", c=D),
                    in_=vt_loc[:, st * D:(st + 1) * D])
            nc.gpsimd.collective_compute(
                "AllGather", mybir.AluOpType.bypass,
                ins=[vin_b[:].opt()], outs=[vout_b[:].opt()],
                replica_groups=GROUPS)

        # ---- Phase 2: Q projection + RoPE (Wq streamed in 4 quarters) ----
        with tc.tile_pool(name="qin", bufs=1) as qin, \
             tc.tile_pool(name="psq", bufs=4, space="PSUM") as psq, \
             tc.tile_pool(name="ropeq", bufs=2) as ropeq:
            xq_sb = qin.tile([P, NCT * SB], bf, name="xq_sb")
            nc.sync.dma_start(out=sinq[:], in_=sinq_d[:, :])
            nc.scalar.dma_start(out=cosq[:], in_=cosq_d[:, :])
            for g in range(4):
                eng = nc.sync if g % 2 == 0 else nc.scalar
                eng.dma_start(out=xq_sb[:, g * 4 * SB:(g + 1) * 4 * SB],
                              in_=xq_d[g])
            for quarter in range(4):
                wqq = wqp.tile([P, NCT * SB], bf, tag="wqq", name=f"wqq_{quarter}")
                eng = nc.sync if quarter % 2 == 0 else nc.scalar
                eng.dma_start(out=wqq[:, :], in_=wq_d[quarter])
                for hh in range(2):
                    h = quarter * 2 + hh
                    pq1 = psq.tile([P, SB], f32, tag="pq", name=f"pq1_{h}")
                    pq2 = psq.tile([P, SB], f32, tag="pq", name=f"pq2_{h}")
                    for ct in range(NCT):
                        nc.tensor.matmul(pq1[:],
                                         wqq[:, ct * SB + 2 * hh * P:ct * SB + (2 * hh + 1) * P],
                                         xq_sb[:, ct * SB:(ct + 1) * SB],
                                         start=(ct == 0), stop=(ct == NCT - 1))
                    for ct in range(NCT):
                        nc.tensor.matmul(pq2[:],
                                         wqq[:, ct * SB + (2 * hh + 1) * P:ct * SB + (2 * hh + 2) * P],
                                         xq_sb[:, ct * SB:(ct + 1) * SB],
                                         start=(ct == 0), stop=(ct == NCT - 1))
                    _rope(nc, ropeq, f32, pq1, pq2, sinq[:], cosq[:],
                          q_sb[:, 2 * h * SB:(2 * h + 1) * SB],
                          q_sb[:, (2 * h + 1) * SB:(2 * h + 2) * SB], SB, f"q{h}")

            # ring-tail unpacks of the gathered K/V shards (each waits on
            # the collective; nothing streams behind them except Wo)
            for j in range(4):
                for dt in range(2):
                    eng = nc.sync if dt == 0 else nc.scalar
                    eng.dma_start(
                        out=k_sb[:, dt * S + j * SB:dt * S + (j + 1) * SB],
                        in_=kout_b[(2 * j + dt) * KP:(2 * j + dt + 1) * KP]
                            .rearrange("(p c) -> p c", c=SB))
            for gst in range(16):
                eng = nc.sync if gst % 2 == 0 else nc.scalar
                eng.dma_start(
                    out=vt_sb[:, gst * VTW:gst * VTW + D],
                    in_=vout_b[gst * VP:(gst + 1) * VP].rearrange("(p c) -> p c", c=D))
        wqp.release()

        # ---- Phase 3+4: attention, software-pipelined by one head ----
        with tc.tile_pool(name="wop", bufs=2) as wop, \
             tc.tile_pool(name="attnp", bufs=1) as attnp:
            attnT = attnp.tile([P, 4 * H * D], bf, name="attnT")
            attn = attnp.tile([P, 16 * SB], bf, name="attn")
            wo_tiles = []
            for half in range(2):
                woh = wop.tile([P, NCT * 1024], bf, tag="woh", name=f"woh_{half}")
                wo_tiles.append(woh)
                eng = nc.sync if half % 2 == 0 else nc.scalar
                eng.dma_start(out=woh[:, :], in_=wo_d[half])

            attention_pools = (
                tc.tile_pool(name="expp", bufs=2),
                tc.tile_pool(name="pss", bufs=2, space="PSUM"),
                tc.tile_pool(name="psa", bufs=2, space="PSUM"),
                tc.tile_pool(name="pst", bufs=2, space="PSUM"),
                tc.tile_pool(name="smallp", bufs=4),
            )
            attn_es = ExitStack()
            expp, pss, psa, pst, smallp = (attn_es.enter_context(p)
                                           for p in attention_pools)

            exp_tiles = {}

            def scores_head(h):
                q0 = q_sb[:, 2 * h * SB:(2 * h + 1) * SB]
                q1 = q_sb[:, (2 * h + 1) * SB:(2 * h + 2) * SB]
                expT = expp.tile([P, 16 * SB], bf, tag="expT", name=f"expT_{h}")
                exp_tiles[h] = expT
                for kp in range(8):  # pairs of k-tiles -> one [128,1024] exp
                    ps = pss.tile([P, 2 * SB], f32, tag="ps", name=f"ps_{h}_{kp}")
                    for i in range(2):
                        kt = 2 * kp + i
                        nc.tensor.matmul(ps[:, i * SB:(i + 1) * SB],
                                         k_sb[:, kt * P:(kt + 1) * P], q0,
                                         start=True, stop=False)
                        nc.tensor.matmul(ps[:, i * SB:(i + 1) * SB],
                                         k_sb[:, S + kt * P:S + (kt + 1) * P], q1,
                                         start=False, stop=True)
                    nc.scalar.activation(expT[:, 2 * kp * SB:2 * (kp + 1) * SB],
                                         ps[:], Exp)

            def attnout_head(h):
                expT = exp_tiles.pop(h)
                for qt in range(4):
                    pa = psa.tile([P, VTW], f32, tag="pa", name=f"pa_{h}_{qt}")
                    for kt in range(16):
                        nc.tensor.matmul(pa[:],
                                         expT[:, kt * SB + qt * P:kt * SB + (qt + 1) * P],
                                         vt_sb[:, kt * VTW:(kt + 1) * VTW],
                                         start=(kt == 0), stop=(kt == 15))
                    rcp = smallp.tile([P, 1], f32, tag="rcp", name=f"rcp_{h}_{qt}")
                    nc.vector.reciprocal(rcp[:], pa[:, D:D + 1])
                    nc.vector.tensor_scalar_mul(
                        attnT[:, qt * H * D + h * D:qt * H * D + (h + 1) * D],
                        pa[:, 0:D], rcp[:])
                # transpose this head's attnT tiles into attn ([c', q])
                for qt in range(4):
                    for c2 in (2 * h, 2 * h + 1):
                        ptr = pst.tile([P, P], bf, tag="ptr", name=f"ptr_{h}_{qt}_{c2}")
                        nc.tensor.transpose(
                            ptr[:],
                            attnT[:, qt * H * D + c2 * P:qt * H * D + (c2 + 1) * P],
                            ident[:])
                        nc.vector.tensor_copy(
                            attn[:, c2 * SB + qt * P:c2 * SB + (qt + 1) * P], ptr[:])

            scores_head(0)
            for h in range(1, H):
                scores_head(h)
                attnout_head(h - 1)
            attnout_head(H - 1)
            attn_es.close()  # free attention PSUM banks before phase 5

            # ---- Phase 5: output projection ----
            with tc.tile_pool(name="pso", bufs=2, space="PSUM") as pso, \
                 tc.tile_pool(name="outp", bufs=3) as outp:
                for half in range(2):
                    woh = wo_tiles[half]
                    for oi in range(8):
                        ot = half * 8 + oi
                        po = pso.tile([P, SB], f32, tag="po", name=f"po_{ot}")
                        for c2 in range(NCT):
                            nc.tensor.matmul(
                                po[:],
                                woh[:, c2 * 1024 + oi * P:c2 * 1024 + (oi + 1) * P],
                                attn[:, c2 * SB:(c2 + 1) * SB],
                                start=(c2 == 0), stop=(c2 == 15))
                        osb = outp.tile([P, SB], f32, tag="osb", name=f"osb_{ot}")
                        nc.scalar.copy(osb[:], po[:])
                        nc.sync.dma_start(out=out_d[ot * P:(ot + 1) * P, :],
                                          in_=osb[:])

    nc.compile()
    return nc


def _get_nc():
    if "nc" not in _CACHE:
        _CACHE["nc"] = _build()
    return _CACHE["nc"]


def make_in_maps(inputs):
    Xq = np.asarray(inputs["Xq"], np.float32)
    Xkv = np.asarray(inputs["Xkv"], np.float32)
    sin_q = np.asarray(inputs["sin_q"], np.float32)
    cos_q = np.asarray(inputs["cos_q"], np.float32)
    sin_k = np.asarray(inputs["sin_k"], np.float32)
    cos_k = np.asarray(inputs["cos_k"], np.float32)
    Wq = np.asarray(inputs["Wq"], np.float32)
    Wk = np.asarray(inputs["Wk"], np.float32)
    Wv = np.asarray(inputs["Wv"], np.float32)
    Wo = np.asarray(inputs["Wo"], np.float32)
    # attn_mask is all zeros by construction (spec fill=zeros) -> no-op.

    scale = np.float32(1.0) / np.sqrt(np.float32(D))

    def img(mat2d, groups):
        """[T*128, W] -> [groups, 128, (T/groups)*W] SBUF-image tiling."""
        rows, w = mat2d.shape
        t = rows // P
        x = mat2d.reshape(t, P, w).transpose(1, 0, 2).reshape(P, t * w)
        gw = t * w // groups
        return np.ascontiguousarray(
            x.reshape(P, groups, gw).transpose(1, 0, 2))

    wqT_f = np.ascontiguousarray(Wq.T).astype(BF16)
    wq_img = np.stack([img(np.ascontiguousarray(wqT_f[:, q * SB:(q + 1) * SB]), 1)[0]
                       for q in range(4)])
    wk_img = img(np.ascontiguousarray(Wk.T).astype(BF16), 1)[0]
    wv_img = img(np.ascontiguousarray(Wv.T).astype(BF16), 1)[0]
    woT_f = np.ascontiguousarray(Wo.T).astype(BF16)
    wo_img = np.stack([img(np.ascontiguousarray(woT_f[:, h * 1024:(h + 1) * 1024]), 1)[0]
                       for h in range(2)])
    xq_bf = Xq.astype(BF16)
    xkv_bf = Xkv.astype(BF16)
    sinq_s = sin_q * scale
    cosq_s = cos_q * scale

    in_maps = []
    for core in range(8):
        b, j = divmod(core, 4)
        sl = slice(j * SB, (j + 1) * SB)
        in_maps.append({
            "xq": img(np.ascontiguousarray(xq_bf[b][:, sl]), 4),
            "xkv": img(np.ascontiguousarray(xkv_bf[b][:, sl]), 4),
            "wqT": wq_img, "wkT": wk_img, "wvT": wv_img, "woT": wo_img,
            "sinq": np.ascontiguousarray(sinq_s[b, 0][:, sl]),
            "cosq": np.ascontiguousarray(cosq_s[b, 0][:, sl]),
            "sink": np.ascontiguousarray(sin_k[b, 0][:, sl]),
            "cosk": np.ascontiguousarray(cos_k[b, 0][:, sl]),
        })
    return in_maps


def kernel(**inputs):
    import time

    from concourse.bass_utils import run_bass_kernel_spmd

    nc = _get_nc()
    in_maps = make_in_maps(inputs)
    res = None
    last_err = None
    for attempt in range(3):
        try:
            res = run_bass_kernel_spmd(nc, in_maps, core_ids=list(range(8)))
            break
        except Exception as e:  # transient NRT/device flakes -- retry
            last_err = e
            time.sleep(3.0)
    if res is None:
        raise last_err
    out = np.empty((B, HID, S), np.float32)
    for core in range(8):
        b, j = divmod(core, 4)
        out[b][:, j * SB:(j + 1) * SB] = res.results[core]["out"]
    return out
